# revision 11
# baseline (speedup 1.0000x reference)
# PointNet++ (3x set-abstraction) forward on 8 Trainium2 NeuronCores.
# Data-parallel over batch: core i processes cloud i. BatchNorm batch stats
# are shared across cores via AllReduce collectives inside the kernel.
import numpy as np
import contextlib
from functools import lru_cache

import concourse.bacc as bacc
import concourse.bass as bass
import concourse.mybir as mybir
from concourse import tile
from concourse.bass_utils import run_bass_kernel_spmd

dt = mybir.dt
Alu = mybir.AluOpType
ActF = mybir.ActivationFunctionType
Axis = mybir.AxisListType

B = 8
N0 = 16384
SA_SPECS = [(1024, 0.1, 32), (512, 0.2, 32), (256, 0.4, 16)]
DIMS = [[3, 32, 32], [32 + 3, 64, 64], [64 + 3, 64, 64, 128]]
BN_EPS = 1e-5


def _r2(radius):
    # match jax: python-float radius*radius (f64) demoted to f32 for compare
    return float(np.float32(np.float64(radius) * np.float64(radius)))


def build_program():
    nc = bacc.Bacc("TRN2", target_bir_lowering=False, debug=False, num_devices=B)
    cloud_d = nc.dram_tensor("cloud", [N0, 3], dt.float32, kind="ExternalInput").ap()
    wparams = []  # per stage, per layer: (W_ap, g_ap, b_ap)
    for si, mdims in enumerate(DIMS):
        layers = []
        for li, (ci, co) in enumerate(zip(mdims[:-1], mdims[1:])):
            w = nc.dram_tensor(f"W_{si}_{li}", [ci, co], dt.float32, kind="ExternalInput").ap()
            g = nc.dram_tensor(f"G_{si}_{li}", [co, 1], dt.float32, kind="ExternalInput").ap()
            b = nc.dram_tensor(f"Bt_{si}_{li}", [co, 1], dt.float32, kind="ExternalInput").ap()
            layers.append((w, g, b))
        wparams.append(layers)
    out_d = nc.dram_tensor("out", [128, 256], dt.float32, kind="ExternalOutput").ap()

    # DRAM scratch
    nx_d = [nc.dram_tensor(f"nx{s}", [SA_SPECS[s][0], 3], dt.float32) for s in range(3)]
    flat_d = [nc.dram_tensor(f"flat{s}", [3, SA_SPECS[s][0]], dt.float32) for s in range(2)]
    feat_d = [nc.dram_tensor(f"feat{s}", [SA_SPECS[s][0], DIMS[s][-1]], dt.float32) for s in range(2)]
    # per-BN-layer collective bounce buffers
    bn_in = []
    bn_out = []
    for si, mdims in enumerate(DIMS):
        for li in range(len(mdims) - 1):
            bn_in.append(nc.dram_tensor(f"bnin_{si}_{li}", [128, 2], dt.float32))
            bn_out.append(nc.dram_tensor(f"bnout_{si}_{li}", [128, 2], dt.float32, addr_space="Shared"))

    with tile.TileContext(nc) as tc:
        with contextlib.ExitStack() as ctx:
            con = ctx.enter_context(tc.tile_pool(name="con", bufs=1))   # constants
            big = ctx.enter_context(tc.tile_pool(name="big", bufs=1))   # stage-long tiles
            wk = ctx.enter_context(tc.tile_pool(name="wk", bufs=2))     # working tiles
            ps = ctx.enter_context(tc.tile_pool(name="ps", bufs=4, space="PSUM"))
            psm = ctx.enter_context(tc.tile_pool(name="psm", bufs=3, space="PSUM"))

            f32 = dt.float32

            # ---- constants ----
            ones_r = con.tile([1, 128], f32)
            nc.vector.memset(ones_r[:], 1.0)
            ones3c = con.tile([128, 3], f32)
            nc.vector.memset(ones3c[:], 1.0)
            # identity for PE transpose
            icol_i = con.tile([128, 1], dt.int32)
            nc.gpsimd.iota(icol_i[:], [[0, 1]], base=0, channel_multiplier=1)
            irow_i = con.tile([128, 128], dt.int32)
            nc.gpsimd.iota(irow_i[:], [[1, 128]], base=0, channel_multiplier=0)
            icol_f = con.tile([128, 1], f32)
            nc.vector.tensor_copy(icol_f[:], icol_i[:])
            irow_f = con.tile([128, 128], f32)
            nc.vector.tensor_copy(irow_f[:], irow_i[:])
            ident = con.tile([128, 128], f32)
            nc.vector.tensor_scalar(ident[:], irow_f[:], icol_f[:], None, Alu.is_equal)
            # in_max constants for max_index rounds: row r -> values r*8+1..r*8+8
            mi_const = con.tile([128, 4, 8], f32)
            mi_i = con.tile([128, 4, 8], dt.int32)
            nc.gpsimd.iota(mi_i[:], [[8, 4], [1, 8]], base=1, channel_multiplier=0)
            nc.vector.tensor_copy(mi_const[:], mi_i[:])
            # slot-id constants [128, 32]: value j+1 at col j
            slot_i = con.tile([128, 32], dt.int32)
            nc.gpsimd.iota(slot_i[:], [[1, 32]], base=1, channel_multiplier=0)
            slot_f = con.tile([128, 32], f32)
            nc.vector.tensor_copy(slot_f[:], slot_i[:])

            def transpose_pe(dst_psum, src):
                nc.tensor.matmul(dst_psum, src, ident[: src.shape[0], : src.shape[0]], is_transpose=True)

            # =========================================================
            # FPS over point tiles X,Y,Z [128, F]; S samples
            # writes newxyz rows into nxbuf [1, 3*S] and returns it
            # =========================================================
            def fps(Xt, Yt, Zt, F, S, p0_src_ap):
                mind = big.tile([128, F], f32, tag=f"mind{S}")
                nc.vector.memset(mind[:], 1e10)
                nxbuf = big.tile([1, 3 * S], f32, tag=f"nx{S}")
                # load point 0 coords -> [1,3]
                p3row = big.tile([1, 3], f32, tag=f"p3r{S}")
                nc.sync.dma_start(p3row[:], p0_src_ap)
                nc.vector.tensor_copy(nxbuf[:, 0:3], p3row[:])
                np3row = big.tile([1, 3], f32, tag=f"np3r{S}")
                nc.vector.tensor_scalar_mul(np3row[:], p3row[:], -1.0)
                pc = big.tile([128, 3], f32, tag=f"pc{S}")
                bc0 = psm.tile([128, 3], f32, tag="pstag")
                nc.tensor.matmul(bc0[:], ones_r[:], np3row[:])
                nc.vector.tensor_copy(pc[:], bc0[:])

                s1 = big.tile([128, F], f32, tag=f"s1{S}")
                s2 = big.tile([128, F], f32, tag=f"s2{S}")
                s3 = big.tile([128, F], f32, tag=f"s3{S}")
                junk = big.tile([128, F], f32, tag=f"jk{S}")
                rowmax = big.tile([128, 1], f32, tag=f"rm{S}")
                rowmax3 = big.tile([128, 3], f32, tag=f"rm3{S}")
                rowmask = big.tile([128, F], f32, tag=f"rmk{S}")
                gmax3 = big.tile([3, 1], f32, tag=f"gm3{S}")
                ohrow3 = big.tile([3, 128], f32, tag=f"oh3{S}")
                pcat = big.tile([128, 3], f32, tag=f"pct{S}")
                junk3 = big.tile([3, 128], f32, tag=f"jk3{S}")
                p3col = big.tile([3, 1], f32, tag=f"p3c{S}")

                with tc.For_i(3, 3 * S, 3) as ofs:
                    nc.scalar.activation(s1[:], Xt[:], ActF.Square, bias=pc[:, 0:1], scale=1.0)
                    nc.scalar.activation(s2[:], Yt[:], ActF.Square, bias=pc[:, 1:2], scale=1.0)
                    nc.scalar.activation(s3[:], Zt[:], ActF.Square, bias=pc[:, 2:3], scale=1.0)
                    nc.vector.tensor_tensor(junk[:], s1[:], s2[:], Alu.add)
                    nc.vector.tensor_tensor(junk[:], junk[:], s3[:], Alu.add)
                    nc.vector.tensor_tensor(mind[:], mind[:], junk[:], Alu.min)
                    nc.vector.tensor_reduce(rowmax[:], mind[:], Axis.X, Alu.max)
                    nc.vector.tensor_scalar(rowmax3[:], ones3c[:], rowmax[:], None, Alu.mult)
                    trp = psm.tile([3, 128], f32, tag="pstag")
                    transpose_pe(trp[:], rowmax3[:])
                    nc.vector.tensor_reduce(gmax3[:], trp[:], Axis.X, Alu.max)
                    nc.vector.tensor_scalar(ohrow3[:], trp[:], gmax3[:], None, Alu.is_equal)
                    nc.vector.tensor_scalar(rowmask[:], mind[:], rowmax[:], None, Alu.is_equal)
                    nc.vector.scalar_tensor_tensor(junk[:], Xt[:], 1.0, rowmask[:], Alu.mult, Alu.mult, accum_out=pcat[:, 0:1])
                    nc.vector.scalar_tensor_tensor(junk[:], Yt[:], 1.0, rowmask[:], Alu.mult, Alu.mult, accum_out=pcat[:, 1:2])
                    nc.vector.scalar_tensor_tensor(junk[:], Zt[:], 1.0, rowmask[:], Alu.mult, Alu.mult, accum_out=pcat[:, 2:3])
                    trc = psm.tile([3, 128], f32, tag="pstag")
                    transpose_pe(trc[:], pcat[:])
                    nc.vector.scalar_tensor_tensor(junk3[:], trc[:], 1.0, ohrow3[:], Alu.mult, Alu.mult, accum_out=p3col[:])
                    trp3 = psm.tile([1, 3], f32, tag="pstag")
                    transpose_pe(trp3[:], p3col[:])
                    nc.vector.tensor_copy(p3row[:], trp3[:])
                    nc.vector.tensor_copy(nxbuf[:, bass.ds(ofs, 3)], p3row[:])
                    nc.vector.tensor_scalar_mul(np3row[:], p3row[:], -1.0)
                    bc = psm.tile([128, 3], f32, tag="pstag")
                    nc.tensor.matmul(bc[:], ones_r[:], np3row[:])
                    nc.vector.tensor_copy(pc[:], bc[:])
                return nxbuf

            # =========================================================
            # Ball query: queries from nxq tiles (nqx/nqy/nqz [128, QT]),
            # points from xrep source (DRAM flats [3, N]), chunked.
            # Returns per-qtile idx tiles [128, K] int32 (list).
            # =========================================================
            def ball_query(flatsrc, N, S, K, r2, stg):
                QT = S // 128
                CH = min(N, 4096)
                NCH = N // CH
                # query coord columns (negated) [128, QT]
                nq = big.tile([128, QT, 3], f32, tag=f"nq{stg}")
                nxsrc = nx_d[stg].ap()  # [S, 3]
                qv = nxsrc.rearrange("(qt p) c -> p qt c", p=128)
                nqtmp = big.tile([128, QT, 3], f32, tag=f"nqt{stg}")
                nc.sync.dma_start(nqtmp[:], qv)
                nc.vector.tensor_scalar_mul(nq[:].rearrange("p a c -> p (a c)"),
                                            nqtmp[:].rearrange("p a c -> p (a c)"), -1.0)
                idx_tiles = []
                xr = [big.tile([128, CH], f32, tag=f"xr{c}", name=f"xr{c}") for c in range(3)]
                d2 = big.tile([128, CH], f32, tag="d2")
                dx = big.tile([128, CH], f32, tag="dx")
                cs = big.tile([128, CH], f32, tag="cs")
                zz = big.tile([128, CH], f32, tag="zz")
                nc.vector.memset(zz[:], 0.0)
                for qt in range(QT):
                    idxf = big.tile([128, K], f32, tag=f"idxf{stg}_{qt}")
                    nc.vector.memset(idxf[:], 0.0)
                    carry = big.tile([128, 1], f32, tag=f"carry{stg}_{qt}")
                    nc.vector.memset(carry[:], 0.0)
                    for ch in range(NCH):
                        for c in range(3):
                            src = flatsrc.ap()[c:c + 1, ch * CH:(ch + 1) * CH].to_broadcast((128, CH))
                            nc.sync.dma_start(xr[c][:], src)
                        nc.scalar.activation(d2[:], xr[0][:], ActF.Square, bias=nq[:, qt, 0:1], scale=1.0)
                        nc.scalar.activation(dx[:], xr[1][:], ActF.Square, bias=nq[:, qt, 1:2], scale=1.0)
                        nc.vector.tensor_tensor(d2[:], d2[:], dx[:], Alu.add)
                        nc.scalar.activation(dx[:], xr[2][:], ActF.Square, bias=nq[:, qt, 2:3], scale=1.0)
                        nc.vector.tensor_tensor(d2[:], d2[:], dx[:], Alu.add)
                        nc.vector.tensor_scalar(dx[:], d2[:], r2, None, Alu.is_lt)
                        nc.vector.tensor_tensor_scan(cs[:], dx[:], zz[:], carry[:], Alu.add, Alu.add)
                        cl = wk.tile([128, 1], f32, tag=f"cl{stg}")
                        nc.vector.tensor_copy(cl[:], cs[:, CH - 1:CH])
                        for r in range(K // 8):
                            fnd = wk.tile([128, 8], dt.uint32, tag=f"fnd{stg}")
                            nc.vector.max_index(fnd[:], mi_const[:, r, :], cs[:])
                            fndf = wk.tile([128, 8], f32, tag=f"fndf{stg}")
                            nc.vector.tensor_copy(fndf[:], fnd[:])
                            if ch > 0:
                                nc.vector.tensor_scalar_add(fndf[:], fndf[:], float(ch * CH))
                            v1 = wk.tile([128, 8], f32, tag=f"v1{stg}")
                            v2 = wk.tile([128, 8], f32, tag=f"v2{stg}")
                            nc.vector.tensor_scalar(v1[:], mi_const[:, r, :], carry[:], None, Alu.is_gt)
                            nc.vector.tensor_scalar(v2[:], mi_const[:, r, :], cl[:], None, Alu.is_le)
                            nc.vector.tensor_tensor(v1[:], v1[:], v2[:], Alu.mult)
                            nc.vector.tensor_tensor(v1[:], v1[:], fndf[:], Alu.mult)
                            nc.vector.tensor_tensor(idxf[:, r * 8:(r + 1) * 8], idxf[:, r * 8:(r + 1) * 8], v1[:], Alu.add)
                        nc.vector.tensor_copy(carry[:], cl[:])
                    # pad: slots j with j+1 > total count get first neighbor
                    pmask = wk.tile([128, K], f32, tag=f"pm{stg}")
                    frep = wk.tile([128, K], f32, tag=f"fr{stg}")
                    nc.vector.tensor_scalar(pmask[:], slot_f[:, :K], carry[:], None, Alu.is_gt)
                    nc.vector.memset(frep[:], 0.0)
                    nc.vector.tensor_scalar(frep[:], frep[:], idxf[:, 0:1], None, Alu.add)
                    nc.vector.tensor_tensor(frep[:], frep[:], pmask[:], Alu.mult)
                    nc.vector.tensor_tensor(idxf[:], idxf[:], frep[:], Alu.add)
                    idxi = big.tile([128, K], dt.int32, tag=f"idxi{stg}_{qt}")
                    nc.vector.tensor_copy(idxi[:], idxf[:])
                    idx_tiles.append(idxi)
                return idx_tiles

            # =========================================================
            # Grouping + MLP + BN + ReLU + max-pool for one stage
            # =========================================================
            bn_ix = [0]

            def stage_mlp(stg, idx_tiles, S, K, Cin, layers_w, src_xyz_d, src_feat_d, Ntot_bnk, pooled_out):
                QT = S // 128
                npts = S * K
                Cf = Cin - 3
                KG = 4           # k-values per 512-col chunk
                NCHK = npts // 512
                src_xyz_ap = src_xyz_d if isinstance(src_xyz_d, bass.AP) else src_xyz_d.ap()
                src_feat_ap = None if src_feat_d is None else (src_feat_d if isinstance(src_feat_d, bass.AP) else src_feat_d.ap())
                nlay = len(layers_w)
                h_d = [nc.dram_tensor(f"hd_{stg}_{li}", [layers_w[li][0].shape[1], npts], dt.float32)
                       for li in range(nlay)]
                qpos = big.tile([128, QT, 3], f32, tag=f"qp{stg}", name="qpos")
                nc.sync.dma_start(qpos[:],
                                  nx_d[stg].ap().rearrange("(qt p) c -> p qt c", p=128))
                # ---- layer 1 streamed over chunks, gathers per qtile ----
                C1 = layers_w[0][0].shape[1]
                W1 = wk.tile([Cin, C1], f32, tag=f"W{stg}_0", name="W1")
                if Cf > 0:
                    nc.sync.dma_start(W1[0:Cf, :], layers_w[0][0][3:3 + Cf, :])
                    nc.sync.dma_start(W1[Cf:Cf + 3, :], layers_w[0][0][0:3, :])
                else:
                    nc.sync.dma_start(W1[:], layers_w[0][0][:])
                sums = big.tile([128, 1], f32, tag="bnsum", name="sums")
                sqs = big.tile([128, 1], f32, tag="bnsq", name="sqs")
                nc.vector.memset(sums[:], 0.0)
                nc.vector.memset(sqs[:], 0.0)
                for qt in range(QT):
                    G = wk.tile([128, K, 3], f32, tag="Gg", name="G")
                    for k in range(K):
                        nc.gpsimd.indirect_dma_start(
                            out=G[:, k, :], out_offset=None, in_=src_xyz_ap,
                            in_offset=bass.IndirectOffsetOnAxis(ap=idx_tiles[qt][:, k:k + 1], axis=0))
                    for c in range(3):
                        v = G[:, :, c:c + 1].rearrange("p a b -> p (a b)")
                        nc.vector.tensor_scalar(v, v, qpos[:, qt, c:c + 1], None, Alu.subtract)
                    if Cf > 0:
                        Fg = wk.tile([128, K, Cf], f32, tag="Fgg", name="Fg")
                        for k in range(K):
                            nc.gpsimd.indirect_dma_start(
                                out=Fg[:, k, :], out_offset=None, in_=src_feat_ap,
                                in_offset=bass.IndirectOffsetOnAxis(ap=idx_tiles[qt][:, k:k + 1], axis=0))
                    for kg in range(K // KG):
                        rhs_c = wk.tile([Cin, 512], f32, tag="rhsc", name="rhs_c")
                        for j in range(KG):
                            k = kg * KG + j
                            t1 = psm.tile([3, 128], f32, tag="pstag", name="t1")
                            transpose_pe(t1[:], G[:, k, :])
                            nc.vector.tensor_copy(rhs_c[Cf:Cf + 3, j * 128:(j + 1) * 128], t1[:])
                            if Cf > 0:
                                t2 = psm.tile([Cf, 128], f32, tag="pstag", name="t2")
                                transpose_pe(t2[:], Fg[:, k, :])
                                nc.vector.tensor_copy(rhs_c[0:Cf, j * 128:(j + 1) * 128], t2[:])
                        pb = ps.tile([C1, 512], f32, tag="mm", name="pb")
                        nc.tensor.matmul(pb[:], W1[:], rhs_c[:])
                        h = wk.tile([C1, 512], f32, tag="hc", name="h")
                        csum = wk.tile([C1, 1], f32, tag="csm", name="csum")
                        nc.vector.tensor_scalar(h[:], pb[:], 1.0, 0.0, Alu.mult, Alu.add, accum_out=csum[:])
                        nc.vector.tensor_tensor(sums[:C1], sums[:C1], csum[:], Alu.add)
                        nc.vector.scalar_tensor_tensor(pb[:], h[:], 1.0, h[:], Alu.mult, Alu.mult, accum_out=csum[:])
                        nc.vector.tensor_tensor(sqs[:C1], sqs[:C1], csum[:], Alu.add)
                        cki = qt * (K // KG) + kg
                        nc.sync.dma_start(h_d[0].ap()[:, cki * 512:(cki + 1) * 512], h[:])

                def bn_reduce(Co, g_ap, b_ap):
                    bi = bn_ix[0]; bn_ix[0] += 1
                    stat = wk.tile([128, 2], f32, tag="st", name="stat")
                    nc.vector.memset(stat[:], 0.0)
                    nc.vector.tensor_copy(stat[:Co, 0:1], sums[:Co])
                    nc.vector.tensor_copy(stat[:Co, 1:2], sqs[:Co])
                    nc.sync.dma_start(bn_in[bi][:, :], stat[:])
                    nc.gpsimd.collective_compute(
                        "AllReduce", Alu.add, replica_groups=[list(range(B))],
                        ins=[bn_in[bi][:, :]], outs=[bn_out[bi][:, :]])
                    gstat = wk.tile([128, 2], f32, tag="gst", name="gstat")
                    nc.sync.dma_start(gstat[:], bn_out[bi][:, :])
                    mean = wk.tile([Co, 1], f32, tag="mean", name="mean")
                    var = wk.tile([Co, 1], f32, tag="var", name="var")
                    inv = 1.0 / float(Ntot_bnk)
                    nc.vector.tensor_scalar_mul(mean[:], gstat[:Co, 0:1], inv)
                    nc.vector.tensor_scalar_mul(var[:], gstat[:Co, 1:2], inv)
                    m2 = wk.tile([Co, 1], f32, tag="m2", name="m2")
                    nc.vector.tensor_tensor(m2[:], mean[:], mean[:], Alu.mult)
                    nc.vector.tensor_tensor(var[:], var[:], m2[:], Alu.subtract)
                    nc.vector.tensor_scalar_add(var[:], var[:], BN_EPS)
                    sd = wk.tile([Co, 1], f32, tag="sd", name="sd")
                    nc.scalar.sqrt(sd[:], var[:])
                    rstd = wk.tile([Co, 1], f32, tag="rs", name="rstd")
                    nc.vector.reciprocal(rstd[:], sd[:])
                    gt = wk.tile([Co, 1], f32, tag="gt", name="gt")
                    bt = wk.tile([Co, 1], f32, tag="bt", name="bt")
                    nc.sync.dma_start(gt[:], g_ap[:])
                    nc.sync.dma_start(bt[:], b_ap[:])
                    aa = big.tile([128, 1], f32, tag="aa", name="aa")
                    bb = big.tile([128, 1], f32, tag="bb", name="bb")
                    nc.vector.tensor_tensor(aa[:Co], rstd[:], gt[:], Alu.mult)
                    nc.vector.scalar_tensor_tensor(bb[:Co], mean[:], -1.0, aa[:Co], Alu.mult, Alu.mult)
                    nc.vector.tensor_tensor(bb[:Co], bb[:Co], bt[:], Alu.add)
                    return aa, bb

                aa, bb = bn_reduce(C1, layers_w[0][1], layers_w[0][2])
                Cprev = C1
                for li in range(1, nlay):
                    Co = layers_w[li][0].shape[1]
                    Wt = wk.tile([Cprev, Co], f32, tag=f"W{stg}_{li}", name="Wt")
                    nc.sync.dma_start(Wt[:], layers_w[li][0][:])
                    nc.vector.memset(sums[:], 0.0)
                    nc.vector.memset(sqs[:], 0.0)
                    for cki in range(NCHK):
                        yc = wk.tile([Cprev, 512], f32, tag="yc", name="yc")
                        nc.sync.dma_start(yc[:], h_d[li - 1].ap()[:, cki * 512:(cki + 1) * 512])
                        nc.scalar.activation(yc[:], yc[:], ActF.Relu, bias=bb[:Cprev], scale=aa[:Cprev])
                        pb = ps.tile([Co, 512], f32, tag="mm", name="pb")
                        nc.tensor.matmul(pb[:], Wt[:], yc[:])
                        h = wk.tile([Co, 512], f32, tag="hc", name="h")
                        csum = wk.tile([Co, 1], f32, tag="csm", name="csum")
                        nc.vector.tensor_scalar(h[:], pb[:], 1.0, 0.0, Alu.mult, Alu.add, accum_out=csum[:])
                        nc.vector.tensor_tensor(sums[:Co], sums[:Co], csum[:], Alu.add)
                        nc.vector.scalar_tensor_tensor(pb[:], h[:], 1.0, h[:], Alu.mult, Alu.mult, accum_out=csum[:])
                        nc.vector.tensor_tensor(sqs[:Co], sqs[:Co], csum[:], Alu.add)
                        nc.sync.dma_start(h_d[li].ap()[:, cki * 512:(cki + 1) * 512], h[:])
                    aa, bb = bn_reduce(Co, layers_w[li][1], layers_w[li][2])
                    Cprev = Co
                # final: relu + pool accumulate
                Cl = Cprev
                nc.vector.memset(pooled_out[:], -1e30)
                for cki in range(NCHK):
                    yc = wk.tile([Cl, 512], f32, tag="yc2", name="yc")
                    nc.sync.dma_start(yc[:], h_d[nlay - 1].ap()[:, cki * 512:(cki + 1) * 512])
                    nc.scalar.activation(yc[:], yc[:], ActF.Relu, bias=bb[:Cl], scale=aa[:Cl])
                    qt = cki // (K // KG)
                    part = wk.tile([Cl, 128], f32, tag="pp", name="part")
                    nc.vector.tensor_reduce(part[:], yc[:].rearrange("c (k p) -> c p k", p=128), Axis.X, Alu.max)
                    po = pooled_out[:, qt * 128:(qt + 1) * 128]
                    nc.vector.tensor_tensor(po, po, part[:], Alu.max)

            # =========================================================
            # Stage 1
            # =========================================================
            X1 = big.tile([128, 128], f32, tag="X1")
            Y1 = big.tile([128, 128], f32, tag="Y1")
            Z1 = big.tile([128, 128], f32, tag="Z1")
            cl3 = cloud_d.rearrange("(p f) c -> p f c", p=128)
            nc.sync.dma_start(X1[:], cl3[:, :, 0:1].rearrange("p f one -> p (f one)"))
            nc.sync.dma_start(Y1[:], cl3[:, :, 1:2].rearrange("p f one -> p (f one)"))
            nc.sync.dma_start(Z1[:], cl3[:, :, 2:3].rearrange("p f one -> p (f one)"))
            # flats for stage-1 ball query xrep: [3, N0]
            flat0 = nc.dram_tensor("flatN0", [3, N0], dt.float32)
            nc.sync.dma_start(flat0.ap()[0, :].rearrange("(p f) -> p f", p=128), X1[:])
            nc.sync.dma_start(flat0.ap()[1, :].rearrange("(p f) -> p f", p=128), Y1[:])
            nc.sync.dma_start(flat0.ap()[2, :].rearrange("(p f) -> p f", p=128), Z1[:])

            nx1 = fps(X1, Y1, Z1, 128, 1024, cloud_d[0:1, 0:3])
            nc.sync.dma_start(nx_d[0].ap()[:, :].rearrange("s c -> (s c)")[None, :], nx1[:])

            idx1 = ball_query(flat0, N0, 1024, 32, _r2(0.1), 0)
            pooled1 = big.tile([32, 1024], f32, tag="pool1")
            stage_mlp(0, idx1, 1024, 32, 3, wparams[0], cloud_d, None,
                      B * 1024 * 32, pooled1)
            # write feats pair-major [1024, 32] via transposes
            for blk in range(8):
                t = psm.tile([128, 32], f32, tag="pstag")
                transpose_pe(t[:], pooled1[:, blk * 128:(blk + 1) * 128])
                st = wk.tile([128, 32], f32, tag="ft1s")
                nc.vector.tensor_copy(st[:], t[:])
                nc.sync.dma_start(feat_d[0].ap()[blk * 128:(blk + 1) * 128, :], st[:])

            # =========================================================
            # Stage 2
            # =========================================================
            X2 = big.tile([128, 8], f32, tag="X2")
            Y2 = big.tile([128, 8], f32, tag="Y2")
            Z2 = big.tile([128, 8], f32, tag="Z2")
            r1 = nx_d[0].ap().rearrange("(p f) c -> p f c", p=128)
            nc.sync.dma_start(X2[:], r1[:, :, 0:1].rearrange("p f one -> p (f one)"))
            nc.sync.dma_start(Y2[:], r1[:, :, 1:2].rearrange("p f one -> p (f one)"))
            nc.sync.dma_start(Z2[:], r1[:, :, 2:3].rearrange("p f one -> p (f one)"))
            nc.sync.dma_start(flat_d[0].ap()[0, :].rearrange("(p f) -> p f", p=128), X2[:])
            nc.sync.dma_start(flat_d[0].ap()[1, :].rearrange("(p f) -> p f", p=128), Y2[:])
            nc.sync.dma_start(flat_d[0].ap()[2, :].rearrange("(p f) -> p f", p=128), Z2[:])

            nx2 = fps(X2, Y2, Z2, 8, 512, nx_d[0].ap()[0:1, 0:3])
            nc.sync.dma_start(nx_d[1].ap()[:, :].rearrange("s c -> (s c)")[None, :], nx2[:])

            idx2 = ball_query(flat_d[0], 1024, 512, 32, _r2(0.2), 1)
            pooled2 = big.tile([64, 512], f32, tag="pool2")
            stage_mlp(1, idx2, 512, 32, 35, wparams[1], nx_d[0], feat_d[0],
                      B * 512 * 32, pooled2)
            for blk in range(4):
                t = psm.tile([128, 64], f32, tag="pstag")
                transpose_pe(t[:], pooled2[:, blk * 128:(blk + 1) * 128])
                st = wk.tile([128, 64], f32, tag="ft2s")
                nc.vector.tensor_copy(st[:], t[:])
                nc.sync.dma_start(feat_d[1].ap()[blk * 128:(blk + 1) * 128, :], st[:])

            # =========================================================
            # Stage 3
            # =========================================================
            X3 = big.tile([128, 4], f32, tag="X3")
            Y3 = big.tile([128, 4], f32, tag="Y3")
            Z3 = big.tile([128, 4], f32, tag="Z3")
            r2v = nx_d[1].ap().rearrange("(p f) c -> p f c", p=128)
            nc.sync.dma_start(X3[:], r2v[:, :, 0:1].rearrange("p f one -> p (f one)"))
            nc.sync.dma_start(Y3[:], r2v[:, :, 1:2].rearrange("p f one -> p (f one)"))
            nc.sync.dma_start(Z3[:], r2v[:, :, 2:3].rearrange("p f one -> p (f one)"))
            nc.sync.dma_start(flat_d[1].ap()[0, :].rearrange("(p f) -> p f", p=128), X3[:])
            nc.sync.dma_start(flat_d[1].ap()[1, :].rearrange("(p f) -> p f", p=128), Y3[:])
            nc.sync.dma_start(flat_d[1].ap()[2, :].rearrange("(p f) -> p f", p=128), Z3[:])

            nx3 = fps(X3, Y3, Z3, 4, 256, nx_d[1].ap()[0:1, 0:3])
            nc.sync.dma_start(nx_d[2].ap()[:, :].rearrange("s c -> (s c)")[None, :], nx3[:])

            idx3 = ball_query(flat_d[1], 512, 256, 16, _r2(0.4), 2)
            pooled3 = big.tile([128, 256], f32, tag="pool3")
            stage_mlp(2, idx3, 256, 16, 67, wparams[2], nx_d[1], feat_d[1],
                      B * 256 * 16, pooled3)
            nc.sync.dma_start(out_d[:, :], pooled3[:])

    nc.compile()
    return nc


@lru_cache(maxsize=1)
def _program():
    return build_program()


def kernel(clouds, params):
    clouds = np.asarray(clouds, dtype=np.float32)
    nc = _program()
    in_maps = []
    for b in range(B):
        m = {"cloud": np.ascontiguousarray(clouds[b])}
        for si, layers in enumerate(params):
            for li, (W, g, bt) in enumerate(layers):
                m[f"W_{si}_{li}"] = np.ascontiguousarray(np.asarray(W, np.float32))
                m[f"G_{si}_{li}"] = np.ascontiguousarray(np.asarray(g, np.float32).reshape(-1, 1))
                m[f"Bt_{si}_{li}"] = np.ascontiguousarray(np.asarray(bt, np.float32).reshape(-1, 1))
        in_maps.append(m)
    res = run_bass_kernel_spmd(nc, in_maps, list(range(B)))
    out = np.stack([res.results[b]["out"] for b in range(B)], axis=0)
    return out


# revision 12
# speedup vs baseline: 1.0904x; 1.0904x over previous
# PointNet++ (3x set-abstraction) forward on 8 Trainium2 NeuronCores.
# Data-parallel over batch: core i processes cloud i. BatchNorm batch stats
# are shared across cores via AllReduce collectives inside the kernel.
import numpy as np
import contextlib
from functools import lru_cache

import concourse.bacc as bacc
import concourse.bass as bass
import concourse.mybir as mybir
from concourse import tile
from concourse.bass_utils import run_bass_kernel_spmd

dt = mybir.dt
Alu = mybir.AluOpType
ActF = mybir.ActivationFunctionType
Axis = mybir.AxisListType

B = 8
N0 = 16384
SA_SPECS = [(1024, 0.1, 32), (512, 0.2, 32), (256, 0.4, 16)]
DIMS = [[3, 32, 32], [32 + 3, 64, 64], [64 + 3, 64, 64, 128]]
BN_EPS = 1e-5


def _r2(radius):
    # match jax: python-float radius*radius (f64) demoted to f32 for compare
    return float(np.float32(np.float64(radius) * np.float64(radius)))


def build_program():
    nc = bacc.Bacc("TRN2", target_bir_lowering=False, debug=False, num_devices=B)
    cloud_d = nc.dram_tensor("cloud", [N0, 3], dt.float32, kind="ExternalInput").ap()
    wparams = []  # per stage, per layer: (W_ap, g_ap, b_ap)
    for si, mdims in enumerate(DIMS):
        layers = []
        for li, (ci, co) in enumerate(zip(mdims[:-1], mdims[1:])):
            w = nc.dram_tensor(f"W_{si}_{li}", [ci, co], dt.float32, kind="ExternalInput").ap()
            g = nc.dram_tensor(f"G_{si}_{li}", [co, 1], dt.float32, kind="ExternalInput").ap()
            b = nc.dram_tensor(f"Bt_{si}_{li}", [co, 1], dt.float32, kind="ExternalInput").ap()
            layers.append((w, g, b))
        wparams.append(layers)
    out_d = nc.dram_tensor("out", [128, 256], dt.float32, kind="ExternalOutput").ap()

    # DRAM scratch
    nx_d = [nc.dram_tensor(f"nx{s}", [SA_SPECS[s][0], 3], dt.float32) for s in range(3)]
    flat_d = [nc.dram_tensor(f"flat{s}", [3, SA_SPECS[s][0]], dt.float32) for s in range(2)]
    feat_d = [nc.dram_tensor(f"feat{s}", [SA_SPECS[s][0], DIMS[s][-1]], dt.float32) for s in range(2)]
    # per-BN-layer collective bounce buffers
    bn_in = []
    bn_out = []
    for si, mdims in enumerate(DIMS):
        for li in range(len(mdims) - 1):
            bn_in.append(nc.dram_tensor(f"bnin_{si}_{li}", [128, 2], dt.float32))
            bn_out.append(nc.dram_tensor(f"bnout_{si}_{li}", [128, 2], dt.float32, addr_space="Shared"))

    with tile.TileContext(nc) as tc:
        with contextlib.ExitStack() as ctx:
            con = ctx.enter_context(tc.tile_pool(name="con", bufs=1))   # constants
            big = ctx.enter_context(tc.tile_pool(name="big", bufs=1))   # stage-long tiles
            wk = ctx.enter_context(tc.tile_pool(name="wk", bufs=2))     # working tiles
            ps = ctx.enter_context(tc.tile_pool(name="ps", bufs=4, space="PSUM"))
            psm = ctx.enter_context(tc.tile_pool(name="psm", bufs=3, space="PSUM"))

            f32 = dt.float32

            # ---- constants ----
            ones_r = con.tile([1, 128], f32)
            nc.vector.memset(ones_r[:], 1.0)
            ones3c = con.tile([128, 3], f32)
            nc.vector.memset(ones3c[:], 1.0)
            # identity for PE transpose
            icol_i = con.tile([128, 1], dt.int32)
            nc.gpsimd.iota(icol_i[:], [[0, 1]], base=0, channel_multiplier=1)
            irow_i = con.tile([128, 128], dt.int32)
            nc.gpsimd.iota(irow_i[:], [[1, 128]], base=0, channel_multiplier=0)
            icol_f = con.tile([128, 1], f32)
            nc.vector.tensor_copy(icol_f[:], icol_i[:])
            irow_f = con.tile([128, 128], f32)
            nc.vector.tensor_copy(irow_f[:], irow_i[:])
            ident = con.tile([128, 128], f32)
            nc.vector.tensor_scalar(ident[:], irow_f[:], icol_f[:], None, Alu.is_equal)
            # in_max constants for max_index rounds: row r -> values r*8+1..r*8+8
            mi_const = con.tile([128, 4, 8], f32)
            mi_i = con.tile([128, 4, 8], dt.int32)
            nc.gpsimd.iota(mi_i[:], [[8, 4], [1, 8]], base=1, channel_multiplier=0)
            nc.vector.tensor_copy(mi_const[:], mi_i[:])
            # slot-id constants [128, 32]: value j+1 at col j
            slot_i = con.tile([128, 32], dt.int32)
            nc.gpsimd.iota(slot_i[:], [[1, 32]], base=1, channel_multiplier=0)
            slot_f = con.tile([128, 32], f32)
            nc.vector.tensor_copy(slot_f[:], slot_i[:])

            def transpose_pe(dst_psum, src):
                nc.tensor.matmul(dst_psum, src, ident[: src.shape[0], : src.shape[0]], is_transpose=True)

            # =========================================================
            # FPS over point tiles X,Y,Z [128, F]; S samples
            # writes newxyz rows into nxbuf [1, 3*S] and returns it
            # =========================================================
            def fps(Xt, Yt, Zt, F, S, p0_src_ap):
                mind = big.tile([128, F], f32, tag=f"mind{S}")
                nc.vector.memset(mind[:], 1e10)
                nxbuf = big.tile([1, 3 * S], f32, tag=f"nx{S}")
                # load point 0 coords -> [1,3]
                p3row = big.tile([1, 3], f32, tag=f"p3r{S}")
                nc.sync.dma_start(p3row[:], p0_src_ap)
                nc.vector.tensor_copy(nxbuf[:, 0:3], p3row[:])
                np3row = big.tile([1, 3], f32, tag=f"np3r{S}")
                nc.vector.tensor_scalar_mul(np3row[:], p3row[:], -1.0)
                pc = big.tile([128, 3], f32, tag=f"pc{S}")
                bc0 = psm.tile([128, 3], f32, tag="pstag")
                nc.tensor.matmul(bc0[:], ones_r[:], np3row[:])
                nc.vector.tensor_copy(pc[:], bc0[:])

                s1 = big.tile([128, F], f32, tag=f"s1{S}")
                s2 = big.tile([128, F], f32, tag=f"s2{S}")
                s3 = big.tile([128, F], f32, tag=f"s3{S}")
                junk = big.tile([128, F], f32, tag=f"jk{S}")
                rowmax = big.tile([128, 1], f32, tag=f"rm{S}")
                rowmax3 = big.tile([128, 3], f32, tag=f"rm3{S}")
                rowmask = big.tile([128, F], f32, tag=f"rmk{S}")
                gmax3 = big.tile([3, 1], f32, tag=f"gm3{S}")
                ohrow3 = big.tile([3, 128], f32, tag=f"oh3{S}")
                pcat = big.tile([128, 3], f32, tag=f"pct{S}")
                junk3 = big.tile([3, 128], f32, tag=f"jk3{S}")
                p3col = big.tile([3, 1], f32, tag=f"p3c{S}")

                with tc.For_i(3, 3 * S, 3, staggered_reset=True) as ofs:
                    nc.scalar.activation(s1[:], Xt[:], ActF.Square, bias=pc[:, 0:1], scale=1.0)
                    nc.scalar.activation(s2[:], Yt[:], ActF.Square, bias=pc[:, 1:2], scale=1.0)
                    nc.scalar.activation(s3[:], Zt[:], ActF.Square, bias=pc[:, 2:3], scale=1.0)
                    nc.vector.tensor_tensor(junk[:], s1[:], s2[:], Alu.add)
                    nc.vector.tensor_tensor(junk[:], junk[:], s3[:], Alu.add)
                    nc.vector.tensor_tensor(mind[:], mind[:], junk[:], Alu.min)
                    nc.vector.tensor_reduce(rowmax[:], mind[:], Axis.X, Alu.max)
                    nc.vector.tensor_scalar(rowmax3[:], ones3c[:], rowmax[:], None, Alu.mult)
                    trp = psm.tile([3, 128], f32, tag="pstag")
                    transpose_pe(trp[:], rowmax3[:])
                    nc.vector.tensor_reduce(gmax3[:], trp[:], Axis.X, Alu.max)
                    nc.vector.tensor_scalar(ohrow3[:], trp[:], gmax3[:], None, Alu.is_equal)
                    nc.vector.tensor_scalar(rowmask[:], mind[:], rowmax[:], None, Alu.is_equal)
                    nc.vector.scalar_tensor_tensor(junk[:], Xt[:], 1.0, rowmask[:], Alu.mult, Alu.mult, accum_out=pcat[:, 0:1])
                    nc.vector.scalar_tensor_tensor(junk[:], Yt[:], 1.0, rowmask[:], Alu.mult, Alu.mult, accum_out=pcat[:, 1:2])
                    nc.vector.scalar_tensor_tensor(junk[:], Zt[:], 1.0, rowmask[:], Alu.mult, Alu.mult, accum_out=pcat[:, 2:3])
                    trc = psm.tile([3, 128], f32, tag="pstag")
                    transpose_pe(trc[:], pcat[:])
                    nc.vector.scalar_tensor_tensor(junk3[:], trc[:], 1.0, ohrow3[:], Alu.mult, Alu.mult, accum_out=p3col[:])
                    trp3 = psm.tile([1, 3], f32, tag="pstag")
                    transpose_pe(trp3[:], p3col[:])
                    nc.vector.tensor_copy(p3row[:], trp3[:])
                    nc.vector.tensor_copy(nxbuf[:, bass.ds(ofs, 3)], p3row[:])
                    nc.vector.tensor_scalar_mul(np3row[:], p3row[:], -1.0)
                    bc = psm.tile([128, 3], f32, tag="pstag")
                    nc.tensor.matmul(bc[:], ones_r[:], np3row[:])
                    nc.vector.tensor_copy(pc[:], bc[:])
                return nxbuf

            # =========================================================
            # Ball query: queries from nxq tiles (nqx/nqy/nqz [128, QT]),
            # points from xrep source (DRAM flats [3, N]), chunked.
            # Returns per-qtile idx tiles [128, K] int32 (list).
            # =========================================================
            def ball_query(flatsrc, N, S, K, r2, stg):
                QT = S // 128
                CH = min(N, 4096)
                NCH = N // CH
                # query coord columns (negated) [128, QT]
                nq = big.tile([128, QT, 3], f32, tag=f"nq{stg}")
                nxsrc = nx_d[stg].ap()  # [S, 3]
                qv = nxsrc.rearrange("(qt p) c -> p qt c", p=128)
                nqtmp = big.tile([128, QT, 3], f32, tag=f"nqt{stg}")
                nc.sync.dma_start(nqtmp[:], qv)
                nc.vector.tensor_scalar_mul(nq[:].rearrange("p a c -> p (a c)"),
                                            nqtmp[:].rearrange("p a c -> p (a c)"), -1.0)
                idx_tiles = []
                xr = [big.tile([128, CH], f32, tag=f"xr{c}", name=f"xr{c}") for c in range(3)]
                d2 = big.tile([128, CH], f32, tag="d2")
                dx = big.tile([128, CH], f32, tag="dx")
                cs = big.tile([128, CH], f32, tag="cs")
                zz = big.tile([128, CH], f32, tag="zz")
                nc.vector.memset(zz[:], 0.0)
                for qt in range(QT):
                    idxf = big.tile([128, K], f32, tag=f"idxf{stg}_{qt}")
                    nc.vector.memset(idxf[:], 0.0)
                    carry = big.tile([128, 1], f32, tag=f"carry{stg}_{qt}")
                    nc.vector.memset(carry[:], 0.0)
                    for ch in range(NCH):
                        for c in range(3):
                            src = flatsrc.ap()[c:c + 1, ch * CH:(ch + 1) * CH].to_broadcast((128, CH))
                            nc.sync.dma_start(xr[c][:], src)
                        nc.scalar.activation(d2[:], xr[0][:], ActF.Square, bias=nq[:, qt, 0:1], scale=1.0)
                        nc.scalar.activation(dx[:], xr[1][:], ActF.Square, bias=nq[:, qt, 1:2], scale=1.0)
                        nc.vector.tensor_tensor(d2[:], d2[:], dx[:], Alu.add)
                        nc.scalar.activation(dx[:], xr[2][:], ActF.Square, bias=nq[:, qt, 2:3], scale=1.0)
                        nc.vector.tensor_tensor(d2[:], d2[:], dx[:], Alu.add)
                        nc.vector.tensor_scalar(dx[:], d2[:], r2, None, Alu.is_lt)
                        nc.vector.tensor_tensor_scan(cs[:], dx[:], zz[:], carry[:], Alu.add, Alu.add)
                        cl = wk.tile([128, 1], f32, tag=f"cl{stg}")
                        nc.vector.tensor_copy(cl[:], cs[:, CH - 1:CH])
                        for r in range(K // 8):
                            fnd = wk.tile([128, 8], dt.uint32, tag=f"fnd{stg}")
                            nc.vector.max_index(fnd[:], mi_const[:, r, :], cs[:])
                            fndf = wk.tile([128, 8], f32, tag=f"fndf{stg}")
                            nc.vector.tensor_copy(fndf[:], fnd[:])
                            if ch > 0:
                                nc.vector.tensor_scalar_add(fndf[:], fndf[:], float(ch * CH))
                            v1 = wk.tile([128, 8], f32, tag=f"v1{stg}")
                            v2 = wk.tile([128, 8], f32, tag=f"v2{stg}")
                            nc.vector.tensor_scalar(v1[:], mi_const[:, r, :], carry[:], None, Alu.is_gt)
                            nc.vector.tensor_scalar(v2[:], mi_const[:, r, :], cl[:], None, Alu.is_le)
                            nc.vector.tensor_tensor(v1[:], v1[:], v2[:], Alu.mult)
                            nc.vector.tensor_tensor(v1[:], v1[:], fndf[:], Alu.mult)
                            nc.vector.tensor_tensor(idxf[:, r * 8:(r + 1) * 8], idxf[:, r * 8:(r + 1) * 8], v1[:], Alu.add)
                        nc.vector.tensor_copy(carry[:], cl[:])
                    # pad: slots j with j+1 > total count get first neighbor
                    pmask = wk.tile([128, K], f32, tag=f"pm{stg}")
                    frep = wk.tile([128, K], f32, tag=f"fr{stg}")
                    nc.vector.tensor_scalar(pmask[:], slot_f[:, :K], carry[:], None, Alu.is_gt)
                    nc.vector.memset(frep[:], 0.0)
                    nc.vector.tensor_scalar(frep[:], frep[:], idxf[:, 0:1], None, Alu.add)
                    nc.vector.tensor_tensor(frep[:], frep[:], pmask[:], Alu.mult)
                    nc.vector.tensor_tensor(idxf[:], idxf[:], frep[:], Alu.add)
                    idxi = big.tile([128, K], dt.int32, tag=f"idxi{stg}_{qt}")
                    nc.vector.tensor_copy(idxi[:], idxf[:])
                    idx_tiles.append(idxi)
                return idx_tiles

            # =========================================================
            # Grouping + MLP + BN + ReLU + max-pool for one stage
            # =========================================================
            bn_ix = [0]

            def stage_mlp(stg, idx_tiles, S, K, Cin, layers_w, src_xyz_d, src_feat_d, Ntot_bnk, pooled_out):
                QT = S // 128
                npts = S * K
                Cf = Cin - 3
                KG = 4           # k-values per 512-col chunk
                NCHK = npts // 512
                src_xyz_ap = src_xyz_d if isinstance(src_xyz_d, bass.AP) else src_xyz_d.ap()
                src_feat_ap = None if src_feat_d is None else (src_feat_d if isinstance(src_feat_d, bass.AP) else src_feat_d.ap())
                nlay = len(layers_w)
                h_d = [nc.dram_tensor(f"hd_{stg}_{li}", [layers_w[li][0].shape[1], npts], dt.float32)
                       for li in range(nlay)]
                qpos = big.tile([128, QT, 3], f32, tag=f"qp{stg}", name="qpos")
                nc.sync.dma_start(qpos[:],
                                  nx_d[stg].ap().rearrange("(qt p) c -> p qt c", p=128))
                # ---- layer 1 streamed over chunks, gathers per qtile ----
                C1 = layers_w[0][0].shape[1]
                W1 = wk.tile([Cin, C1], f32, tag=f"W{stg}_0", name="W1")
                if Cf > 0:
                    nc.sync.dma_start(W1[0:Cf, :], layers_w[0][0][3:3 + Cf, :])
                    nc.sync.dma_start(W1[Cf:Cf + 3, :], layers_w[0][0][0:3, :])
                else:
                    nc.sync.dma_start(W1[:], layers_w[0][0][:])
                sums = big.tile([128, 1], f32, tag="bnsum", name="sums")
                sqs = big.tile([128, 1], f32, tag="bnsq", name="sqs")
                nc.vector.memset(sums[:], 0.0)
                nc.vector.memset(sqs[:], 0.0)
                for qt in range(QT):
                    G = wk.tile([128, K, 3], f32, tag="Gg", name="G")
                    for k in range(K):
                        nc.gpsimd.indirect_dma_start(
                            out=G[:, k, :], out_offset=None, in_=src_xyz_ap,
                            in_offset=bass.IndirectOffsetOnAxis(ap=idx_tiles[qt][:, k:k + 1], axis=0))
                    for c in range(3):
                        v = G[:, :, c:c + 1].rearrange("p a b -> p (a b)")
                        nc.vector.tensor_scalar(v, v, qpos[:, qt, c:c + 1], None, Alu.subtract)
                    if Cf > 0:
                        Fg = wk.tile([128, K, Cf], f32, tag="Fgg", name="Fg")
                        for k in range(K):
                            nc.gpsimd.indirect_dma_start(
                                out=Fg[:, k, :], out_offset=None, in_=src_feat_ap,
                                in_offset=bass.IndirectOffsetOnAxis(ap=idx_tiles[qt][:, k:k + 1], axis=0))
                    for kg in range(K // KG):
                        rhs_c = wk.tile([Cin, 512], f32, tag="rhsc", name="rhs_c")
                        for j in range(KG):
                            k = kg * KG + j
                            t1 = psm.tile([3, 128], f32, tag="pstag", name="t1")
                            transpose_pe(t1[:], G[:, k, :])
                            nc.vector.tensor_copy(rhs_c[Cf:Cf + 3, j * 128:(j + 1) * 128], t1[:])
                            if Cf > 0:
                                t2 = psm.tile([Cf, 128], f32, tag="pstag", name="t2")
                                transpose_pe(t2[:], Fg[:, k, :])
                                nc.vector.tensor_copy(rhs_c[0:Cf, j * 128:(j + 1) * 128], t2[:])
                        pb = ps.tile([C1, 512], f32, tag="mm", name="pb")
                        nc.tensor.matmul(pb[:], W1[:], rhs_c[:])
                        h = wk.tile([C1, 512], f32, tag="hc", name="h")
                        csum = wk.tile([C1, 1], f32, tag="csm", name="csum")
                        nc.vector.tensor_scalar(h[:], pb[:], 1.0, 0.0, Alu.mult, Alu.add, accum_out=csum[:])
                        nc.vector.tensor_tensor(sums[:C1], sums[:C1], csum[:], Alu.add)
                        nc.vector.scalar_tensor_tensor(pb[:], h[:], 1.0, h[:], Alu.mult, Alu.mult, accum_out=csum[:])
                        nc.vector.tensor_tensor(sqs[:C1], sqs[:C1], csum[:], Alu.add)
                        cki = qt * (K // KG) + kg
                        nc.sync.dma_start(h_d[0].ap()[:, cki * 512:(cki + 1) * 512], h[:])

                def bn_reduce(Co, g_ap, b_ap):
                    bi = bn_ix[0]; bn_ix[0] += 1
                    stat = wk.tile([128, 2], f32, tag="st", name="stat")
                    nc.vector.memset(stat[:], 0.0)
                    nc.vector.tensor_copy(stat[:Co, 0:1], sums[:Co])
                    nc.vector.tensor_copy(stat[:Co, 1:2], sqs[:Co])
                    nc.sync.dma_start(bn_in[bi][:, :], stat[:])
                    nc.gpsimd.collective_compute(
                        "AllReduce", Alu.add, replica_groups=[list(range(B))],
                        ins=[bn_in[bi][:, :]], outs=[bn_out[bi][:, :]])
                    gstat = wk.tile([128, 2], f32, tag="gst", name="gstat")
                    nc.sync.dma_start(gstat[:], bn_out[bi][:, :])
                    mean = wk.tile([Co, 1], f32, tag="mean", name="mean")
                    var = wk.tile([Co, 1], f32, tag="var", name="var")
                    inv = 1.0 / float(Ntot_bnk)
                    nc.vector.tensor_scalar_mul(mean[:], gstat[:Co, 0:1], inv)
                    nc.vector.tensor_scalar_mul(var[:], gstat[:Co, 1:2], inv)
                    m2 = wk.tile([Co, 1], f32, tag="m2", name="m2")
                    nc.vector.tensor_tensor(m2[:], mean[:], mean[:], Alu.mult)
                    nc.vector.tensor_tensor(var[:], var[:], m2[:], Alu.subtract)
                    nc.vector.tensor_scalar_add(var[:], var[:], BN_EPS)
                    sd = wk.tile([Co, 1], f32, tag="sd", name="sd")
                    nc.scalar.sqrt(sd[:], var[:])
                    rstd = wk.tile([Co, 1], f32, tag="rs", name="rstd")
                    nc.vector.reciprocal(rstd[:], sd[:])
                    gt = wk.tile([Co, 1], f32, tag="gt", name="gt")
                    bt = wk.tile([Co, 1], f32, tag="bt", name="bt")
                    nc.sync.dma_start(gt[:], g_ap[:])
                    nc.sync.dma_start(bt[:], b_ap[:])
                    aa = big.tile([128, 1], f32, tag="aa", name="aa")
                    bb = big.tile([128, 1], f32, tag="bb", name="bb")
                    nc.vector.tensor_tensor(aa[:Co], rstd[:], gt[:], Alu.mult)
                    nc.vector.scalar_tensor_tensor(bb[:Co], mean[:], -1.0, aa[:Co], Alu.mult, Alu.mult)
                    nc.vector.tensor_tensor(bb[:Co], bb[:Co], bt[:], Alu.add)
                    return aa, bb

                aa, bb = bn_reduce(C1, layers_w[0][1], layers_w[0][2])
                Cprev = C1
                for li in range(1, nlay):
                    Co = layers_w[li][0].shape[1]
                    Wt = wk.tile([Cprev, Co], f32, tag=f"W{stg}_{li}", name="Wt")
                    nc.sync.dma_start(Wt[:], layers_w[li][0][:])
                    nc.vector.memset(sums[:], 0.0)
                    nc.vector.memset(sqs[:], 0.0)
                    for cki in range(NCHK):
                        yc = wk.tile([Cprev, 512], f32, tag="yc", name="yc")
                        nc.sync.dma_start(yc[:], h_d[li - 1].ap()[:, cki * 512:(cki + 1) * 512])
                        nc.scalar.activation(yc[:], yc[:], ActF.Relu, bias=bb[:Cprev], scale=aa[:Cprev])
                        pb = ps.tile([Co, 512], f32, tag="mm", name="pb")
                        nc.tensor.matmul(pb[:], Wt[:], yc[:])
                        h = wk.tile([Co, 512], f32, tag="hc", name="h")
                        csum = wk.tile([Co, 1], f32, tag="csm", name="csum")
                        nc.vector.tensor_scalar(h[:], pb[:], 1.0, 0.0, Alu.mult, Alu.add, accum_out=csum[:])
                        nc.vector.tensor_tensor(sums[:Co], sums[:Co], csum[:], Alu.add)
                        nc.vector.scalar_tensor_tensor(pb[:], h[:], 1.0, h[:], Alu.mult, Alu.mult, accum_out=csum[:])
                        nc.vector.tensor_tensor(sqs[:Co], sqs[:Co], csum[:], Alu.add)
                        nc.sync.dma_start(h_d[li].ap()[:, cki * 512:(cki + 1) * 512], h[:])
                    aa, bb = bn_reduce(Co, layers_w[li][1], layers_w[li][2])
                    Cprev = Co
                # final: relu + pool accumulate
                Cl = Cprev
                nc.vector.memset(pooled_out[:], -1e30)
                for cki in range(NCHK):
                    yc = wk.tile([Cl, 512], f32, tag="yc2", name="yc")
                    nc.sync.dma_start(yc[:], h_d[nlay - 1].ap()[:, cki * 512:(cki + 1) * 512])
                    nc.scalar.activation(yc[:], yc[:], ActF.Relu, bias=bb[:Cl], scale=aa[:Cl])
                    qt = cki // (K // KG)
                    part = wk.tile([Cl, 128], f32, tag="pp", name="part")
                    nc.vector.tensor_reduce(part[:], yc[:].rearrange("c (k p) -> c p k", p=128), Axis.X, Alu.max)
                    po = pooled_out[:, qt * 128:(qt + 1) * 128]
                    nc.vector.tensor_tensor(po, po, part[:], Alu.max)

            # =========================================================
            # Stage 1
            # =========================================================
            X1 = big.tile([128, 128], f32, tag="X1")
            Y1 = big.tile([128, 128], f32, tag="Y1")
            Z1 = big.tile([128, 128], f32, tag="Z1")
            cl3 = cloud_d.rearrange("(p f) c -> p f c", p=128)
            nc.sync.dma_start(X1[:], cl3[:, :, 0:1].rearrange("p f one -> p (f one)"))
            nc.sync.dma_start(Y1[:], cl3[:, :, 1:2].rearrange("p f one -> p (f one)"))
            nc.sync.dma_start(Z1[:], cl3[:, :, 2:3].rearrange("p f one -> p (f one)"))
            # flats for stage-1 ball query xrep: [3, N0]
            flat0 = nc.dram_tensor("flatN0", [3, N0], dt.float32)
            nc.sync.dma_start(flat0.ap()[0, :].rearrange("(p f) -> p f", p=128), X1[:])
            nc.sync.dma_start(flat0.ap()[1, :].rearrange("(p f) -> p f", p=128), Y1[:])
            nc.sync.dma_start(flat0.ap()[2, :].rearrange("(p f) -> p f", p=128), Z1[:])

            nx1 = fps(X1, Y1, Z1, 128, 1024, cloud_d[0:1, 0:3])
            nc.sync.dma_start(nx_d[0].ap()[:, :].rearrange("s c -> (s c)")[None, :], nx1[:])

            idx1 = ball_query(flat0, N0, 1024, 32, _r2(0.1), 0)
            pooled1 = big.tile([32, 1024], f32, tag="pool1")
            stage_mlp(0, idx1, 1024, 32, 3, wparams[0], cloud_d, None,
                      B * 1024 * 32, pooled1)
            # write feats pair-major [1024, 32] via transposes
            for blk in range(8):
                t = psm.tile([128, 32], f32, tag="pstag")
                transpose_pe(t[:], pooled1[:, blk * 128:(blk + 1) * 128])
                st = wk.tile([128, 32], f32, tag="ft1s")
                nc.vector.tensor_copy(st[:], t[:])
                nc.sync.dma_start(feat_d[0].ap()[blk * 128:(blk + 1) * 128, :], st[:])

            # =========================================================
            # Stage 2
            # =========================================================
            X2 = big.tile([128, 8], f32, tag="X2")
            Y2 = big.tile([128, 8], f32, tag="Y2")
            Z2 = big.tile([128, 8], f32, tag="Z2")
            r1 = nx_d[0].ap().rearrange("(p f) c -> p f c", p=128)
            nc.sync.dma_start(X2[:], r1[:, :, 0:1].rearrange("p f one -> p (f one)"))
            nc.sync.dma_start(Y2[:], r1[:, :, 1:2].rearrange("p f one -> p (f one)"))
            nc.sync.dma_start(Z2[:], r1[:, :, 2:3].rearrange("p f one -> p (f one)"))
            nc.sync.dma_start(flat_d[0].ap()[0, :].rearrange("(p f) -> p f", p=128), X2[:])
            nc.sync.dma_start(flat_d[0].ap()[1, :].rearrange("(p f) -> p f", p=128), Y2[:])
            nc.sync.dma_start(flat_d[0].ap()[2, :].rearrange("(p f) -> p f", p=128), Z2[:])

            nx2 = fps(X2, Y2, Z2, 8, 512, nx_d[0].ap()[0:1, 0:3])
            nc.sync.dma_start(nx_d[1].ap()[:, :].rearrange("s c -> (s c)")[None, :], nx2[:])

            idx2 = ball_query(flat_d[0], 1024, 512, 32, _r2(0.2), 1)
            pooled2 = big.tile([64, 512], f32, tag="pool2")
            stage_mlp(1, idx2, 512, 32, 35, wparams[1], nx_d[0], feat_d[0],
                      B * 512 * 32, pooled2)
            for blk in range(4):
                t = psm.tile([128, 64], f32, tag="pstag")
                transpose_pe(t[:], pooled2[:, blk * 128:(blk + 1) * 128])
                st = wk.tile([128, 64], f32, tag="ft2s")
                nc.vector.tensor_copy(st[:], t[:])
                nc.sync.dma_start(feat_d[1].ap()[blk * 128:(blk + 1) * 128, :], st[:])

            # =========================================================
            # Stage 3
            # =========================================================
            X3 = big.tile([128, 4], f32, tag="X3")
            Y3 = big.tile([128, 4], f32, tag="Y3")
            Z3 = big.tile([128, 4], f32, tag="Z3")
            r2v = nx_d[1].ap().rearrange("(p f) c -> p f c", p=128)
            nc.sync.dma_start(X3[:], r2v[:, :, 0:1].rearrange("p f one -> p (f one)"))
            nc.sync.dma_start(Y3[:], r2v[:, :, 1:2].rearrange("p f one -> p (f one)"))
            nc.sync.dma_start(Z3[:], r2v[:, :, 2:3].rearrange("p f one -> p (f one)"))
            nc.sync.dma_start(flat_d[1].ap()[0, :].rearrange("(p f) -> p f", p=128), X3[:])
            nc.sync.dma_start(flat_d[1].ap()[1, :].rearrange("(p f) -> p f", p=128), Y3[:])
            nc.sync.dma_start(flat_d[1].ap()[2, :].rearrange("(p f) -> p f", p=128), Z3[:])

            nx3 = fps(X3, Y3, Z3, 4, 256, nx_d[1].ap()[0:1, 0:3])
            nc.sync.dma_start(nx_d[2].ap()[:, :].rearrange("s c -> (s c)")[None, :], nx3[:])

            idx3 = ball_query(flat_d[1], 512, 256, 16, _r2(0.4), 2)
            pooled3 = big.tile([128, 256], f32, tag="pool3")
            stage_mlp(2, idx3, 256, 16, 67, wparams[2], nx_d[1], feat_d[1],
                      B * 256 * 16, pooled3)
            nc.sync.dma_start(out_d[:, :], pooled3[:])

    nc.compile()
    return nc


@lru_cache(maxsize=1)
def _program():
    return build_program()


def kernel(clouds, params):
    clouds = np.asarray(clouds, dtype=np.float32)
    nc = _program()
    in_maps = []
    for b in range(B):
        m = {"cloud": np.ascontiguousarray(clouds[b])}
        for si, layers in enumerate(params):
            for li, (W, g, bt) in enumerate(layers):
                m[f"W_{si}_{li}"] = np.ascontiguousarray(np.asarray(W, np.float32))
                m[f"G_{si}_{li}"] = np.ascontiguousarray(np.asarray(g, np.float32).reshape(-1, 1))
                m[f"Bt_{si}_{li}"] = np.ascontiguousarray(np.asarray(bt, np.float32).reshape(-1, 1))
        in_maps.append(m)
    res = run_bass_kernel_spmd(nc, in_maps, list(range(B)))
    out = np.stack([res.results[b]["out"] for b in range(B)], axis=0)
    return out
